# revision 13
# baseline (speedup 1.0000x reference)
"""AngularDistribution Trainium2 kernel (8 NeuronCores, SPMD data-parallel over (batch,atom) pairs).

Math (per pair p, triple n, offset r, filter f):
  rad[n,r]  = exp(c_r*S1[n] - 12*o_r^2 - g*S2[n])     c_r = 2*g*o_r, S1=rij+rik+rjk, S2=sum sq
  ang[n,f]  = 2*u^z (f<4, u=(1-ct)/2) or 2*v^z (f>=4, v=(1+ct)/2), z in {1,2,4,8}
  cut[n]    = (cos(pi*rij/10)*cos(pi*rik/10)*cos(pi*rjk/10))^2
  out[p,r*8+f] = sum_n rad[n,r] * ang[n,f] * cut[n] * mask[n]

Per core: 64 pairs, nch*128 compacted+padded triples each (padding r=5.0 gives
cut == 0, matching the reference's (r<5) gate). Host pre-permutes inputs into
the exact SBUF layout so every DMA is one contiguous block.

Scheduling: all chunk-scoped data lives in per-chunk tiles so the tile
dependency tracker never serializes chunk c+1's elementwise against chunk c's
contraction. Scalar runs Square/Reciprocal/Sin/Exp with each ACT table loaded
exactly once. GpSimd (high fixed cost per op) gets only a short chain. The
pair stage is software-pipelined (u-matmul h+1 issued before contraction h)
so exp overlaps the contraction instead of ping-ponging with it.
"""

import os
import sys

sys.path.insert(0, "/opt/trn_rl_repo")

import numpy as np
from contextlib import ExitStack

GAMMA = 4.0
N_CORES = 8
PP = 64          # pairs per core (512 total / 8)
R = 32
F = 8
NCHK = 4         # column chunks for the elementwise pipeline

_CACHE = {}
LAST_EXEC_NS = None


def _build(nch):
    import concourse.bass as bass
    import concourse.tile as tile
    from concourse import bacc, mybir

    f32 = mybir.dt.float32
    f32r = mybir.dt.float32r
    bf16 = mybir.dt.bfloat16
    fp16 = mybir.dt.float16
    Alu = mybir.AluOpType
    Act = mybir.ActivationFunctionType
    W = PP * nch          # global free size
    CW = W // NCHK        # chunk cols
    PPC = PP // NCHK      # pairs per chunk (16)
    KR = 2 * nch + 1      # lhsT rows per pair (S1 rows, S2g rows, ones row)
    KRP = 32              # padded row stride: 4 pairs per 128-row block
    NBC = PPC * KRP // 128  # transpose blocks per chunk (4)
    PI = float(np.pi)

    nc = bacc.Bacc("TRN2", target_bir_lowering=False, debug=False,
                   num_devices=N_CORES)

    d_rij = nc.dram_tensor("rij", [NCHK, 128, CW], f32, kind="ExternalInput")
    d_rik = nc.dram_tensor("rik", [NCHK, 128, CW], f32, kind="ExternalInput")
    d_rjk = nc.dram_tensor("rjk", [NCHK, 128, CW], f32, kind="ExternalInput")
    d_bd = nc.dram_tensor("bdiag", [128, nch * R], f32, kind="ExternalInput")
    d_id = nc.dram_tensor("ident", [128, 128], f32, kind="ExternalInput")
    d_sel = nc.dram_tensor("sel", [128, R], f32, kind="ExternalInput")
    d_out = nc.dram_tensor("out", [R, PP * F], f32, kind="ExternalOutput")

    with tile.TileContext(nc) as tc, ExitStack() as ctx:
        cpool = ctx.enter_context(tc.tile_pool(name="consts", bufs=1))
        gpool = ctx.enter_context(tc.tile_pool(name="glob", bufs=1))
        ppool = ctx.enter_context(tc.tile_pool(name="pair", bufs=4))
        pcpool = ctx.enter_context(tc.tile_pool(name="pc", bufs=1, space="PSUM"))
        ps2pool = ctx.enter_context(tc.tile_pool(name="ps2", bufs=1, space="PSUM"))
        pupool = ctx.enter_context(tc.tile_pool(name="psu", bufs=2, space="PSUM"))
        ptpool = ps2pool

        bd_t = cpool.tile([128, nch * R], f32r)
        nc.scalar.dma_start(bd_t[:], d_bd.ap().bitcast(f32r))
        id_t = cpool.tile([128, 128], f32)
        nc.scalar.dma_start(id_t[:], d_id.ap())
        sel_f32 = cpool.tile([128, R], f32)
        nc.gpsimd.dma_start(sel_f32[:], d_sel.ap())
        sel_t = cpool.tile([128, R], bf16)

        bias0 = cpool.tile([128, 1], f32)
        nc.vector.memset(bias0[:], 0.0)
        bias_hpi = cpool.tile([128, 1], f32)
        nc.vector.memset(bias_hpi[:], float(np.pi) / 2.0)

        # ---- per-chunk tiles ----
        qs = [nc.sync, nc.scalar, nc.gpsimd]
        CH = []
        for c in range(NCHK):
            t = {}
            for nm in ("rij", "rik", "rjk", "tij2", "tik2", "tjk2", "s12",
                       "num", "s2a", "s2g", "s1a", "s1", "den", "rden"):
                t[nm] = gpool.tile([128, CW], f32, name=f"{nm}_{c}")
            for nm in ("ct", "u1", "v1", "c1", "c2", "c3", "p12", "p2"):
                t[nm] = gpool.tile([128, CW], fp16, name=f"{nm}_{c}")
            for nm in ("u2", "v2", "u4", "v4", "u8", "v8", "cm"):
                t[nm] = gpool.tile([128, CW], bf16, name=f"{nm}_{c}")
            t["pall"] = gpool.tile([128, F * CW], bf16, name=f"pall_{c}")
            t["src"] = gpool.tile([128, NBC * 128], f32, name=f"src_{c}")
            t["ts"] = gpool.tile([128, NBC * 128], f32r, name=f"ts_{c}")
            CH.append(t)

        # input DMAs spread across queues, emitted before all compute
        for c in range(NCHK):
            t = CH[c]
            for i, (nm, d) in enumerate((("rij", d_rij), ("rik", d_rik),
                                         ("rjk", d_rjk))):
                qs[(3 * c + i) % 3].dma_start(t[nm][:], d.ap()[c])

        nc.vector.tensor_copy(sel_t[:], sel_f32[:])
        for c in range(NCHK):
            src3 = CH[c]["src"][:].rearrange("p (pair k) -> p pair k", k=KRP)
            nc.vector.memset(src3[:, :, 2 * nch:2 * nch + 1], 1.0)

        # ---- scalar phase A: squares, recips, sins (one table load each) ----
        for c in range(NCHK):
            t = CH[c]
            nc.scalar.activation(t["tij2"][:], t["rij"][:], Act.Square, bias=bias0[:])
            nc.scalar.activation(t["tik2"][:], t["rik"][:], Act.Square, bias=bias0[:])
            nc.scalar.activation(t["tjk2"][:], t["rjk"][:], Act.Square, bias=bias0[:])
        for c in range(NCHK):
            t = CH[c]
            nc.scalar.activation(t["c1"][:], t["rij"][:], Act.Sin, scale=PI / 10.0, bias=bias_hpi[:])
            nc.scalar.activation(t["c2"][:], t["rik"][:], Act.Sin, scale=PI / 10.0, bias=bias_hpi[:])
            nc.scalar.activation(t["c3"][:], t["rjk"][:], Act.Sin, scale=PI / 10.0, bias=bias_hpi[:])

        # ---- V/G elementwise per chunk ----
        def glob_chunk(c):
            t = CH[c]
            V, G = nc.vector, nc.gpsimd
            V.tensor_tensor(t["den"][:], t["rij"][:], t["rik"][:], Alu.mult)
            V.reciprocal_approx_fast(t["rden"][:], t["den"][:])
            V.tensor_tensor(t["s12"][:], t["tij2"][:], t["tik2"][:], Alu.add)
            V.scalar_tensor_tensor(t["num"][:], t["tjk2"][:], -1.0, t["s12"][:],
                                   Alu.mult, Alu.add)
            V.tensor_tensor(t["s2a"][:], t["s12"][:], t["tjk2"][:], Alu.add)
            V.tensor_scalar(t["s2g"][:], t["s2a"][:], GAMMA, None, Alu.mult)
            V.tensor_tensor(t["s1a"][:], t["rij"][:], t["rik"][:], Alu.add)
            V.tensor_tensor(t["s1"][:], t["s1a"][:], t["rjk"][:], Alu.add)
            V.scalar_tensor_tensor(t["ct"][:], t["num"][:], 0.5, t["rden"][:],
                                   Alu.mult, Alu.mult)
            V.tensor_scalar(t["u1"][:], t["ct"][:], -0.5, 0.5, Alu.mult, Alu.add)
            V.tensor_scalar(t["v1"][:], t["u1"][:], -1.0, 1.0, Alu.mult, Alu.add)
            V.tensor_tensor(t["u2"][:], t["u1"][:], t["u1"][:], Alu.mult)
            V.tensor_tensor(t["v2"][:], t["v1"][:], t["v1"][:], Alu.mult)
            V.tensor_tensor(t["u4"][:], t["u2"][:], t["u2"][:], Alu.mult)
            V.tensor_tensor(t["v4"][:], t["v2"][:], t["v2"][:], Alu.mult)
            V.tensor_tensor(t["u8"][:], t["u4"][:], t["u4"][:], Alu.mult)
            V.tensor_tensor(t["v8"][:], t["v4"][:], t["v4"][:], Alu.mult)
            # gpsimd: short independent cut chain + two pall products
            G.tensor_tensor(t["p12"][:], t["c1"][:], t["c2"][:], Alu.mult)
            G.tensor_tensor(t["p2"][:], t["p12"][:], t["c3"][:], Alu.mult)
            # cm = p2^2 (the reference's factor 2 is folded into sel)
            G.tensor_tensor(t["cm"][:], t["p2"][:], t["p2"][:], Alu.mult)
            for fi, pw in enumerate(("u1", "u2", "u4", "u8", "v1", "v2", "v4", "v8")):
                eng = G if fi in (3, 7) else V
                eng.tensor_tensor(t["pall"][:, fi * CW:(fi + 1) * CW],
                                  t[pw][:], t["cm"][:], Alu.mult)
            # exponent source rows
            src3 = t["src"][:].rearrange("p (pair k) -> p pair k", k=KRP)
            s1_3 = t["s1"][:].rearrange("p (pair j) -> p pair j", j=nch)
            s2_3 = t["s2g"][:].rearrange("p (pair j) -> p pair j", j=nch)
            V.tensor_copy(src3[:, :, 0:nch], s1_3[:])
            V.tensor_copy(src3[:, :, nch:2 * nch], s2_3[:])

        def chunk_transposes(c):
            t = CH[c]
            for b in range(NBC):
                pst = ptpool.tile([128, 128], f32, name=f"pst{c}_{b}", tag="pst")
                nc.tensor.transpose(pst[:], t["src"][:, b * 128:(b + 1) * 128], id_t[:])
                nc.vector.tensor_copy(t["ts"][:, b * 128:(b + 1) * 128], pst[:])

        outs_t = gpool.tile([R, PP * F], f32)
        bchunks = [[j for j in range(nch) if j % 4 == b] for b in range(4)]

        def umm_exp(g, h):
            """u-matmuls + exp for pair sub-block h of group g; returns rad."""
            c = g // 2
            ts = CH[c]["ts"]
            psu = pupool.tile([128, 1024], f32, name=f"psu{g}_{h}", tag="psu")
            rad = ppool.tile([128, 2 * nch * R], bf16, name=f"rad{g}_{h}",
                             tag="rad", bufs=6)
            for e in range(2):
                lp = (g % 2) * 32 + h * 2 + e      # pair local to chunk... (see below)
                pair = g * 8 + h * 2 + e
                lpc = pair - c * PPC               # pair local to chunk c
                blk, p0 = lpc // 4, 32 * (lpc % 4)
                nc.tensor.matmul(psu[:, e * 512:e * 512 + nch * R],
                                 ts[p0:p0 + KR, blk * 128:(blk + 1) * 128],
                                 bd_t[p0:p0 + KR, :],
                                 start=True, stop=True, tile_position=(p0, 0))
            nc.scalar.activation(
                rad[:].rearrange("p (e c) -> p e c", e=2),
                psu[:].rearrange("p (e c) -> p e c", e=2)[:, :, 0:nch * R],
                Act.Exp, bias=bias0[:])
            return rad

        def contraction(g, h, rad, pc):
            c = g // 2
            pall_sc = CH[c]["pall"][:].rearrange("p (f col) -> p col f", f=F)
            for e in range(2):
                pair = g * 8 + h * 2 + e
                lpc = pair - c * PPC
                q = h * 2 + e
                for b in range(4):
                    for ji, j in enumerate(bchunks[b]):
                        nc.tensor.matmul(
                            pc[32 * b:32 * b + 32, q * F:(q + 1) * F],
                            rad[:, (e * nch + j) * R:(e * nch + j + 1) * R],
                            pall_sc[:, lpc * nch + j, :],
                            start=(ji == 0), stop=(ji == len(bchunks[b]) - 1),
                            tile_position=(0, 32 * b),
                        )

        def group_tail(g, pc):
            sb = ppool.tile([128, 8 * F], bf16, name=f"sb{g}", tag="sb")
            nc.vector.tensor_copy(sb[:], pc[:])
            ps2 = ps2pool.tile([R, 8 * F], f32, name=f"ps2_{g}", tag="pst")
            nc.tensor.matmul(ps2[:], sel_t[:], sb[:], start=True, stop=True)
            nc.vector.tensor_copy(outs_t[:, g * 8 * F:(g + 1) * 8 * F], ps2[:])
            nc.sync.dma_start(d_out.ap()[:, g * 8 * F:(g + 1) * 8 * F],
                              outs_t[:, g * 8 * F:(g + 1) * 8 * F])

        for c in range(NCHK):
            glob_chunk(c)

        # pair stage, software-pipelined one sub-block ahead
        for g in range(PP // 8):
            if g % 2 == 0:
                chunk_transposes(g // 2)
            pc = pcpool.tile([128, 8 * F], f32, name=f"pc{g}", tag="pc")
            rads = [umm_exp(g, 0)]
            for h in range(4):
                if h < 3:
                    rads.append(umm_exp(g, h + 1))
                contraction(g, h, rads[h], pc)
            group_tail(g, pc)

    nc.compile()
    return nc


def _prep(r_ij, r_ik, r_jk, offsets, triple_masks):
    """Host-side shard + compact + pad + pre-permute. Returns (in_maps, nch)."""
    B, A, N = r_ij.shape
    P = B * A
    rij = np.ascontiguousarray(r_ij, dtype=np.float32).reshape(P, N)
    rik = np.ascontiguousarray(r_ik, dtype=np.float32).reshape(P, N)
    rjk = np.ascontiguousarray(r_jk, dtype=np.float32).reshape(P, N)
    m = (np.asarray(triple_masks).reshape(P, N) != 0)

    counts = m.sum(axis=1)
    npad = max(128, int(-(-max(1, counts.max()) // 128) * 128))
    nch = npad // 128
    W = PP * nch
    CW = W // NCHK

    cij = np.full((P, npad), 5.0, dtype=np.float32)
    cik = np.full((P, npad), 5.0, dtype=np.float32)
    cjk = np.full((P, npad), 5.0, dtype=np.float32)
    for p in range(P):
        idx = np.nonzero(m[p])[0]
        k = idx.size
        cij[p, :k] = rij[p, idx]
        cik[p, :k] = rik[p, idx]
        cjk[p, :k] = rjk[p, idx]

    o = np.asarray(offsets, dtype=np.float32)
    c32 = (2.0 * GAMMA * o).astype(np.float32)       # c_r
    b32 = (-3.0 * GAMMA * o * o).astype(np.float32)  # -12*o_r^2

    bd = np.zeros((128, nch * R), dtype=np.float32)
    for g in range(4):
        for j in range(nch):
            bd[32 * g + j, j * R:(j + 1) * R] = c32
            bd[32 * g + nch + j, j * R:(j + 1) * R] = -1.0
            bd[32 * g + 2 * nch, j * R:(j + 1) * R] = b32
    ident = np.eye(128, dtype=np.float32)
    # factor 2 here: ang = 2^{1-z}(1 +- ct)^z = 2*u^z, and cm carries p2^2 only
    sel = np.tile(2.0 * np.eye(R, dtype=np.float32), (4, 1))

    def dev_layout(x):
        # x [PP, npad] -> [NCHK, 128, CW]: device col (pair*nch + j) on
        # partition p holds triple (j*128 + p) of that pair
        t = x.reshape(PP, nch, 128).transpose(2, 0, 1).reshape(128, W)
        return np.ascontiguousarray(
            t.reshape(128, NCHK, CW).transpose(1, 0, 2))

    in_maps = []
    for c in range(N_CORES):
        lo, hi = c * PP, (c + 1) * PP
        in_maps.append({
            "rij": dev_layout(cij[lo:hi]), "rik": dev_layout(cik[lo:hi]),
            "rjk": dev_layout(cjk[lo:hi]),
            "bdiag": bd, "ident": ident, "sel": sel,
        })
    return in_maps, nch


def _ensure_ntff_hook():
    """Register the axon NTFF profile hook if the image's antenv lacks it."""
    import types
    try:
        from antenv.axon_hooks import get_axon_ntff_profile_hook  # noqa: F401
        return
    except ImportError:
        pass
    try:
        sys.path.insert(0, "/root/.axon_site")
        from trn_agent_boot.trn_boot import _ntff_profile_via_ctypes
        hook = _ntff_profile_via_ctypes("/opt/axon/libaxon_pjrt.so")
        import antenv
        mod = types.ModuleType("antenv.axon_hooks")
        _holder = {"h": hook}
        mod.set_axon_ntff_profile_hook = lambda h: _holder.update(h=h)
        mod.get_axon_ntff_profile_hook = lambda: _holder["h"]
        sys.modules["antenv.axon_hooks"] = mod
        antenv.axon_hooks = mod
    except Exception:
        pass


def kernel(r_ij, r_ik, r_jk, offsets, triple_masks):
    global LAST_EXEC_NS
    from concourse.bass_utils import run_bass_kernel_spmd
    _ensure_ntff_hook()

    B, A, N = r_ij.shape
    in_maps, nch = _prep(r_ij, r_ik, r_jk, offsets, triple_masks)
    if nch not in _CACHE:
        _CACHE[nch] = _build(nch)
    nc = _CACHE[nch]

    trace = os.environ.get("KERNEL_TRACE", "0") == "1"
    res = run_bass_kernel_spmd(nc, in_maps, core_ids=list(range(N_CORES)),
                               trace=trace)
    LAST_EXEC_NS = res.exec_time_ns
    # device out is [R, PP*F]; un-transpose to [PP, R*F] per core
    outs = []
    for r in res.results:
        o = r["out"].reshape(R, PP, F).transpose(1, 0, 2).reshape(PP, R * F)
        outs.append(o)
    out = np.concatenate(outs, axis=0)
    return out.reshape(B, A, R * F)


# revision 20
# speedup vs baseline: 1.3319x; 1.3319x over previous
"""AngularDistribution Trainium2 kernel (8 NeuronCores, SPMD data-parallel over (batch,atom) pairs).

Math (per pair p, triple n, offset r, filter f):
  rad[n,r]  = exp(c_r*S1[n] - 12*o_r^2 - g*S2[n])     c_r = 2*g*o_r, S1=rij+rik+rjk, S2=sum sq
  ang[n,f]  = 2*u^z (f<4, u=(1-ct)/2) or 2*v^z (f>=4, v=(1+ct)/2), z in {1,2,4,8}
  cut[n]    = (cos(pi*rij/10)*cos(pi*rik/10)*cos(pi*rjk/10))^2
  out[p,r*8+f] = sum_n rad[n,r] * ang[n,f] * cut[n] * mask[n]

Per core: 64 pairs, nch*128 compacted+padded triples each (padding r=5.0 gives
cut == 0, matching the reference's (r<5) gate). Host pre-permutes inputs into
the exact SBUF layout so every DMA is one contiguous block.

Scheduling: all chunk-scoped data lives in per-chunk tiles so the tile
dependency tracker never serializes chunk c+1's elementwise against chunk c's
contraction. Scalar runs Square/Reciprocal/Sin/Exp with each ACT table loaded
exactly once. GpSimd (high fixed cost per op) gets only a short chain. The
pair stage is software-pipelined (u-matmul h+1 issued before contraction h)
so exp overlaps the contraction instead of ping-ponging with it.
"""

import os
import sys

sys.path.insert(0, "/opt/trn_rl_repo")

import numpy as np
from contextlib import ExitStack

GAMMA = 4.0
N_CORES = 8
PP = 64          # pairs per core (512 total / 8)
R = 32
F = 8
NCHK = 2         # column chunks for the elementwise pipeline

_CACHE = {}
LAST_EXEC_NS = None


def _build(nch):
    import concourse.bass as bass
    import concourse.tile as tile
    from concourse import bacc, mybir

    f32 = mybir.dt.float32
    f32r = mybir.dt.float32r
    bf16 = mybir.dt.bfloat16
    fp16 = mybir.dt.float16
    Alu = mybir.AluOpType
    Act = mybir.ActivationFunctionType
    W = PP * nch          # global free size
    CW = W // NCHK        # chunk cols
    PPC = PP // NCHK      # pairs per chunk (16)
    KR = 2 * nch + 1      # lhsT rows per pair (S1 rows, S2g rows, ones row)
    KRP = 32              # padded row stride: 4 pairs per 128-row block
    NBC = PPC * KRP // 128  # transpose blocks per chunk (4)
    PI = float(np.pi)

    nc = bacc.Bacc("TRN2", target_bir_lowering=False, debug=False,
                   num_devices=N_CORES)

    d_rij = nc.dram_tensor("rij", [NCHK, 128, CW], f32, kind="ExternalInput")
    d_rik = nc.dram_tensor("rik", [NCHK, 128, CW], f32, kind="ExternalInput")
    d_rjk = nc.dram_tensor("rjk", [NCHK, 128, CW], f32, kind="ExternalInput")
    d_bd = nc.dram_tensor("bdiag", [128, nch * R], f32, kind="ExternalInput")
    d_id = nc.dram_tensor("ident", [128, 128], f32, kind="ExternalInput")
    d_sel = nc.dram_tensor("sel", [128, R], f32, kind="ExternalInput")
    d_out = nc.dram_tensor("out", [R, PP * F], f32, kind="ExternalOutput")

    with tile.TileContext(nc) as tc, ExitStack() as ctx:
        cpool = ctx.enter_context(tc.tile_pool(name="consts", bufs=1))
        gpool = ctx.enter_context(tc.tile_pool(name="glob", bufs=1))
        ppool = ctx.enter_context(tc.tile_pool(name="pair", bufs=4))
        pcpool = ctx.enter_context(tc.tile_pool(name="pc", bufs=1, space="PSUM"))
        ps2pool = ctx.enter_context(tc.tile_pool(name="ps2", bufs=1, space="PSUM"))
        pupool = ctx.enter_context(tc.tile_pool(name="psu", bufs=2, space="PSUM"))
        ptpool = ps2pool

        bd_t = cpool.tile([128, nch * R], f32r)
        nc.scalar.dma_start(bd_t[:], d_bd.ap().bitcast(f32r))
        id_t = cpool.tile([128, 128], f32)
        nc.scalar.dma_start(id_t[:], d_id.ap())
        sel_f32 = cpool.tile([128, R], f32)
        nc.gpsimd.dma_start(sel_f32[:], d_sel.ap())
        sel_t = cpool.tile([128, R], bf16)

        bias0 = cpool.tile([128, 1], f32)
        nc.vector.memset(bias0[:], 0.0)
        bias_hpi = cpool.tile([128, 1], f32)
        nc.vector.memset(bias_hpi[:], float(np.pi) / 2.0)

        # ---- per-chunk tiles ----
        qs = [nc.sync, nc.scalar, nc.gpsimd]
        CH = []
        for c in range(NCHK):
            t = {}
            for nm in ("rij", "rik", "rjk", "tij2", "tik2", "tjk2", "s12",
                       "num", "s2a", "s2g", "s1a", "s1", "den", "rden"):
                t[nm] = gpool.tile([128, CW], f32, name=f"{nm}_{c}")
            for nm in ("ct", "u1", "v1", "c1", "c2", "c3", "p12", "p2"):
                t[nm] = gpool.tile([128, CW], fp16, name=f"{nm}_{c}")
            for nm in ("u2", "v2", "u4", "v4", "u8", "v8", "cm"):
                t[nm] = gpool.tile([128, CW], bf16, name=f"{nm}_{c}")
            t["pall"] = gpool.tile([128, F * CW], bf16, name=f"pall_{c}")
            t["src"] = gpool.tile([128, NBC * 128], f32, name=f"src_{c}")
            t["ts"] = gpool.tile([128, NBC * 128], f32r, name=f"ts_{c}")
            CH.append(t)

        # input DMAs spread across queues, emitted before all compute
        for c in range(NCHK):
            t = CH[c]
            for i, (nm, d) in enumerate((("rij", d_rij), ("rik", d_rik),
                                         ("rjk", d_rjk))):
                qs[(3 * c + i) % 3].dma_start(t[nm][:], d.ap()[c])

        nc.vector.tensor_copy(sel_t[:], sel_f32[:])
        for c in range(NCHK):
            src3 = CH[c]["src"][:].rearrange("p (pair k) -> p pair k", k=KRP)
            nc.vector.memset(src3[:, :, 2 * nch:2 * nch + 1], 1.0)

        # ---- scalar phase A: squares, recips, sins (one table load each) ----
        for c in range(NCHK):
            t = CH[c]
            nc.scalar.activation(t["tij2"][:], t["rij"][:], Act.Square, bias=bias0[:])
            nc.scalar.activation(t["tik2"][:], t["rik"][:], Act.Square, bias=bias0[:])
            nc.scalar.activation(t["tjk2"][:], t["rjk"][:], Act.Square, bias=bias0[:])
        for c in range(NCHK):
            t = CH[c]
            nc.scalar.activation(t["c1"][:], t["rij"][:], Act.Sin, scale=PI / 10.0, bias=bias_hpi[:])
            nc.scalar.activation(t["c2"][:], t["rik"][:], Act.Sin, scale=PI / 10.0, bias=bias_hpi[:])
            nc.scalar.activation(t["c3"][:], t["rjk"][:], Act.Sin, scale=PI / 10.0, bias=bias_hpi[:])

        # ---- V/G elementwise per chunk ----
        def glob_chunk(c):
            t = CH[c]
            V, G = nc.vector, nc.gpsimd
            V.tensor_tensor(t["den"][:], t["rij"][:], t["rik"][:], Alu.mult)
            V.reciprocal_approx_fast(t["rden"][:], t["den"][:])
            V.tensor_tensor(t["s12"][:], t["tij2"][:], t["tik2"][:], Alu.add)
            V.scalar_tensor_tensor(t["num"][:], t["tjk2"][:], -1.0, t["s12"][:],
                                   Alu.mult, Alu.add)
            V.tensor_tensor(t["s2a"][:], t["s12"][:], t["tjk2"][:], Alu.add)
            V.tensor_scalar(t["s2g"][:], t["s2a"][:], GAMMA, None, Alu.mult)
            V.tensor_tensor(t["s1a"][:], t["rij"][:], t["rik"][:], Alu.add)
            V.tensor_tensor(t["s1"][:], t["s1a"][:], t["rjk"][:], Alu.add)
            V.scalar_tensor_tensor(t["ct"][:], t["num"][:], 0.5, t["rden"][:],
                                   Alu.mult, Alu.mult)
            V.tensor_scalar(t["u1"][:], t["ct"][:], -0.5, 0.5, Alu.mult, Alu.add)
            V.tensor_scalar(t["v1"][:], t["u1"][:], -1.0, 1.0, Alu.mult, Alu.add)
            V.tensor_tensor(t["u2"][:], t["u1"][:], t["u1"][:], Alu.mult)
            V.tensor_tensor(t["v2"][:], t["v1"][:], t["v1"][:], Alu.mult)
            V.tensor_tensor(t["u4"][:], t["u2"][:], t["u2"][:], Alu.mult)
            V.tensor_tensor(t["v4"][:], t["v2"][:], t["v2"][:], Alu.mult)
            V.tensor_tensor(t["u8"][:], t["u4"][:], t["u4"][:], Alu.mult)
            V.tensor_tensor(t["v8"][:], t["v4"][:], t["v4"][:], Alu.mult)
            # gpsimd: short independent cut chain + two pall products
            G.tensor_tensor(t["p12"][:], t["c1"][:], t["c2"][:], Alu.mult)
            G.tensor_tensor(t["p2"][:], t["p12"][:], t["c3"][:], Alu.mult)
            # cm = p2^2 (the reference's factor 2 is folded into sel)
            G.tensor_tensor(t["cm"][:], t["p2"][:], t["p2"][:], Alu.mult)
            for fi, pw in enumerate(("u1", "u2", "u4", "u8", "v1", "v2", "v4", "v8")):
                eng = G if fi in (3, 7) else V
                eng.tensor_tensor(t["pall"][:, fi * CW:(fi + 1) * CW],
                                  t[pw][:], t["cm"][:], Alu.mult)
            # exponent source rows (gpsimd: V must stay off the exponent
            # critical path or phase B stalls behind all of phase A)
            src3 = t["src"][:].rearrange("p (pair k) -> p pair k", k=KRP)
            s1_3 = t["s1"][:].rearrange("p (pair j) -> p pair j", j=nch)
            s2_3 = t["s2g"][:].rearrange("p (pair j) -> p pair j", j=nch)
            G.tensor_copy(src3[:, :, 0:nch], s1_3[:])
            G.tensor_copy(src3[:, :, nch:2 * nch], s2_3[:])

        def chunk_transposes(c):
            # gpsimd cannot read PSUM; the pst->ts copies go on V, emitted at
            # points where V is not mid-elementwise
            t = CH[c]
            for b in range(NBC):
                pst = ptpool.tile([128, 128], f32, name=f"pst{c}_{b}", tag="pst")
                nc.tensor.transpose(pst[:], t["src"][:, b * 128:(b + 1) * 128], id_t[:])
                nc.vector.tensor_copy(t["ts"][:, b * 128:(b + 1) * 128], pst[:])

        outs_t = gpool.tile([R, PP * F], f32)
        bchunks = [[j for j in range(nch) if j % 4 == b] for b in range(4)]

        GPC = (PP // 8) // NCHK  # groups per chunk

        def umm_exp(g, h):
            """u-matmuls + exp for pair sub-block h of group g; returns rad."""
            c = g // GPC
            ts = CH[c]["ts"]
            psu = pupool.tile([128, 1024], f32, name=f"psu{g}_{h}", tag="psu")
            rad = ppool.tile([128, 2 * nch * R], bf16, name=f"rad{g}_{h}",
                             tag="rad", bufs=6)
            for e in range(2):
                pair = g * 8 + h * 2 + e
                lpc = pair - c * PPC               # pair local to chunk c
                blk, p0 = lpc // 4, 32 * (lpc % 4)
                nc.tensor.matmul(psu[:, e * 512:e * 512 + nch * R],
                                 ts[p0:p0 + KR, blk * 128:(blk + 1) * 128],
                                 bd_t[p0:p0 + KR, :],
                                 start=True, stop=True, tile_position=(p0, 0))
            nc.scalar.activation(
                rad[:].rearrange("p (e c) -> p e c", e=2),
                psu[:].rearrange("p (e c) -> p e c", e=2)[:, :, 0:nch * R],
                Act.Exp, bias=bias0[:])
            return rad

        def contraction(g, h, rad, pc):
            c = g // GPC
            pall_sc = CH[c]["pall"][:].rearrange("p (f col) -> p col f", f=F)
            for e in range(2):
                pair = g * 8 + h * 2 + e
                lpc = pair - c * PPC
                q = h * 2 + e
                for b in range(4):
                    for ji, j in enumerate(bchunks[b]):
                        nc.tensor.matmul(
                            pc[32 * b:32 * b + 32, q * F:(q + 1) * F],
                            rad[:, (e * nch + j) * R:(e * nch + j + 1) * R],
                            pall_sc[:, lpc * nch + j, :],
                            start=(ji == 0), stop=(ji == len(bchunks[b]) - 1),
                            tile_position=(0, 32 * b),
                        )

        def group_tail(g, pc):
            sb = ppool.tile([128, 8 * F], bf16, name=f"sb{g}", tag="sb")
            nc.vector.tensor_copy(sb[:], pc[:])
            ps2 = ps2pool.tile([R, 8 * F], f32, name=f"ps2_{g}", tag="pst")
            nc.tensor.matmul(ps2[:], sel_t[:], sb[:], start=True, stop=True)
            nc.vector.tensor_copy(outs_t[:, g * 8 * F:(g + 1) * 8 * F], ps2[:])
            qs[g % 3].dma_start(d_out.ap()[:, g * 8 * F:(g + 1) * 8 * F],
                                outs_t[:, g * 8 * F:(g + 1) * 8 * F])

        def pair_group(g):
            pc = pcpool.tile([128, 8 * F], f32, name=f"pc{g}", tag="pc")
            rads = [umm_exp(g, 0)]
            for h in range(4):
                if h < 3:
                    rads.append(umm_exp(g, h + 1))
                contraction(g, h, rads[h], pc)
            group_tail(g, pc)

        # chunk 0 elementwise, its transposes (V does the ts copies right
        # after finishing chunk 0), chunk 1 elementwise, then chunk-0 groups;
        # chunk-1 transposes land when V is free again, before its groups
        glob_chunk(0)
        chunk_transposes(0)
        glob_chunk(1)
        for g in range(GPC):
            pair_group(g)
        chunk_transposes(1)
        for g in range(GPC, 2 * GPC):
            pair_group(g)

    nc.compile()
    return nc


def _prep(r_ij, r_ik, r_jk, offsets, triple_masks):
    """Host-side shard + compact + pad + pre-permute. Returns (in_maps, nch)."""
    B, A, N = r_ij.shape
    P = B * A
    rij = np.ascontiguousarray(r_ij, dtype=np.float32).reshape(P, N)
    rik = np.ascontiguousarray(r_ik, dtype=np.float32).reshape(P, N)
    rjk = np.ascontiguousarray(r_jk, dtype=np.float32).reshape(P, N)
    m = (np.asarray(triple_masks).reshape(P, N) != 0)

    counts = m.sum(axis=1)
    npad = max(128, int(-(-max(1, counts.max()) // 128) * 128))
    nch = npad // 128
    W = PP * nch
    CW = W // NCHK

    cij = np.full((P, npad), 5.0, dtype=np.float32)
    cik = np.full((P, npad), 5.0, dtype=np.float32)
    cjk = np.full((P, npad), 5.0, dtype=np.float32)
    for p in range(P):
        idx = np.nonzero(m[p])[0]
        k = idx.size
        cij[p, :k] = rij[p, idx]
        cik[p, :k] = rik[p, idx]
        cjk[p, :k] = rjk[p, idx]

    o = np.asarray(offsets, dtype=np.float32)
    c32 = (2.0 * GAMMA * o).astype(np.float32)       # c_r
    b32 = (-3.0 * GAMMA * o * o).astype(np.float32)  # -12*o_r^2

    bd = np.zeros((128, nch * R), dtype=np.float32)
    for g in range(4):
        for j in range(nch):
            bd[32 * g + j, j * R:(j + 1) * R] = c32
            bd[32 * g + nch + j, j * R:(j + 1) * R] = -1.0
            bd[32 * g + 2 * nch, j * R:(j + 1) * R] = b32
    ident = np.eye(128, dtype=np.float32)
    # factor 2 here: ang = 2^{1-z}(1 +- ct)^z = 2*u^z, and cm carries p2^2 only
    sel = np.tile(2.0 * np.eye(R, dtype=np.float32), (4, 1))

    def dev_layout(x):
        # x [PP, npad] -> [NCHK, 128, CW]: device col (pair*nch + j) on
        # partition p holds triple (j*128 + p) of that pair
        t = x.reshape(PP, nch, 128).transpose(2, 0, 1).reshape(128, W)
        return np.ascontiguousarray(
            t.reshape(128, NCHK, CW).transpose(1, 0, 2))

    in_maps = []
    for c in range(N_CORES):
        lo, hi = c * PP, (c + 1) * PP
        in_maps.append({
            "rij": dev_layout(cij[lo:hi]), "rik": dev_layout(cik[lo:hi]),
            "rjk": dev_layout(cjk[lo:hi]),
            "bdiag": bd, "ident": ident, "sel": sel,
        })
    return in_maps, nch


def _ensure_ntff_hook():
    """Register the axon NTFF profile hook if the image's antenv lacks it."""
    import types
    try:
        from antenv.axon_hooks import get_axon_ntff_profile_hook  # noqa: F401
        return
    except ImportError:
        pass
    try:
        sys.path.insert(0, "/root/.axon_site")
        from trn_agent_boot.trn_boot import _ntff_profile_via_ctypes
        hook = _ntff_profile_via_ctypes("/opt/axon/libaxon_pjrt.so")
        import antenv
        mod = types.ModuleType("antenv.axon_hooks")
        _holder = {"h": hook}
        mod.set_axon_ntff_profile_hook = lambda h: _holder.update(h=h)
        mod.get_axon_ntff_profile_hook = lambda: _holder["h"]
        sys.modules["antenv.axon_hooks"] = mod
        antenv.axon_hooks = mod
    except Exception:
        pass


def kernel(r_ij, r_ik, r_jk, offsets, triple_masks):
    global LAST_EXEC_NS
    from concourse.bass_utils import run_bass_kernel_spmd
    _ensure_ntff_hook()

    B, A, N = r_ij.shape
    in_maps, nch = _prep(r_ij, r_ik, r_jk, offsets, triple_masks)
    if nch not in _CACHE:
        _CACHE[nch] = _build(nch)
    nc = _CACHE[nch]

    trace = os.environ.get("KERNEL_TRACE", "0") == "1"
    res = run_bass_kernel_spmd(nc, in_maps, core_ids=list(range(N_CORES)),
                               trace=trace)
    LAST_EXEC_NS = res.exec_time_ns
    # device out is [R, PP*F]; un-transpose to [PP, R*F] per core
    outs = []
    for r in res.results:
        o = r["out"].reshape(R, PP, F).transpose(1, 0, 2).reshape(PP, R * F)
        outs.append(o)
    out = np.concatenate(outs, axis=0)
    return out.reshape(B, A, R * F)
